# revision 18
# baseline (speedup 1.0000x reference)
"""Differential multi-head attention on 8 Trainium2 NeuronCores.

Sharding: core p owns head pair (p, p+8) for both batches (tensor parallel
over the 8 differential head pairs). lambda scalars are folded into the
output-projection weights on the host. Host sums the 8 partial outputs.

v2 schedule: the ACT engine's exp stream (128 x [128,1024] activations,
~142us) is the hard floor; everything else is laid out to keep it gapless:
 - flat (group, kc) software pipeline: scores(i) -> exp(i) -> PV(i-1)
 - projections split into ~0.9us units, EDF-placed as PE filler inside the
   attention loop (b1's K/V prefetched during b0's ACT-bound groups)
 - softmax denominators: DVE reciprocal straight from PSUM row 64, then
   GpSimd partition_broadcast (no DRAM round trip)
 - all output-projection work deferred to b1 groups + epilogue
 - PE warmup dummies during the initial DMA wait (HAM un-throttle)
"""
import numpy as np

import concourse.bacc as bacc
import concourse.bass as bass
import concourse.tile as tile
import concourse.mybir as mybir
from concourse import library_config
from concourse.bass_utils import run_bass_kernel_spmd

F32 = mybir.dt.float32
F16 = mybir.dt.float16

EMBED = 1024
H2 = 8
HD = 64
B = 2
N = 2048
T = B * N  # 4096
NCORES = 8
LAMBDA_INIT = 0.8
SCALE = HD ** -0.5

TRACE = False
LAST_RESULT = [None]

_compiled = [None]


def ts(i, size):
    return slice(i * size, (i + 1) * size)


def _build():
    nc = bacc.Bacc("TRN2", target_bir_lowering=False, debug=False, num_devices=NCORES)

    xT_d = nc.dram_tensor("xT", [128, 8, 8, 512], F16, kind="ExternalInput").ap()
    wq_d = nc.dram_tensor("wq", [128, 8, 128], F16, kind="ExternalInput").ap()
    wk_d = nc.dram_tensor("wk", [128, 8, 128], F16, kind="ExternalInput").ap()
    wv_d = nc.dram_tensor("wv", [128, 8, 128], F16, kind="ExternalInput").ap()
    wc_d = nc.dram_tensor("wcomb", [128, 1024], F16, kind="ExternalInput").ap()
    bq_d = nc.dram_tensor("bq", [128, 1], F32, kind="ExternalInput").ap()
    bk_d = nc.dram_tensor("bk", [128, 1], F32, kind="ExternalInput").ap()
    bva_d = nc.dram_tensor("bvaug", [1, 130], F32, kind="ExternalInput").ap()
    outT_d = nc.dram_tensor("outT", [EMBED, T], F16, kind="ExternalOutput").ap()

    with tile.TileContext(nc) as tc:
        with (
            tc.tile_pool(name="consts", bufs=1) as consts,
            tc.tile_pool(name="xp", bufs=8) as xp,
            tc.tile_pool(name="qkv", bufs=1) as qkv,
            tc.tile_pool(name="ptp", bufs=8) as ptp,
            tc.tile_pool(name="stage", bufs=3) as stage,
            tc.tile_pool(name="normp", bufs=2) as normp,
            tc.tile_pool(name="outp", bufs=4) as outp,
            tc.tile_pool(name="ps_st", bufs=2, space="PSUM") as ps_st,
            tc.tile_pool(name="ps_ot", bufs=1, space="PSUM") as ps_ot,
            tc.tile_pool(name="ps_c", bufs=2, space="PSUM") as ps_c,
        ):
            # ---- gpsimd: switch to the attn library (partition_broadcast) ----
            nc.gpsimd.load_library(library_config.attn)

            # ---- constant / input tiles ----
            wq_t = consts.tile([128, 8, 128], F16, name="wq_t")
            wk_t = consts.tile([128, 8, 128], F16, name="wk_t")
            wv_t = consts.tile([128, 8, 128], F16, name="wv_t")
            wc_t = consts.tile([128, 1024], F16, name="wc_t")
            bq_t = consts.tile([128, 1], F32, name="bq_t")
            bk_t = consts.tile([128, 1], F32, name="bk_t")
            bva_t = consts.tile([128, 130], F32, name="bva_t")
            dum_t = consts.tile([128, 64], F16, name="dum_t")

            qt_t = qkv.tile([128, T], F16, name="qt_t")
            kt_t = qkv.tile([128, T], F16, name="kt_t")
            v_t = qkv.tile([128, 32, 200], F16, name="v_t")
            ot_t = qkv.tile([128, B, N], F16, name="ot_t")
            oc_t = qkv.tile([128, B, N], F16, name="oc_t")

            xt_tiles = {}

            def xt_fetch(t, eng=None, half=None):
                if half is None:
                    xt = xp.tile([128, 8, 512], F16, name="xt")
                    eng.dma_start(out=xt, in_=xT_d[:, t, :, :])
                    xt_tiles[t] = xt
                    return
                if half == 0:
                    xt = xp.tile([128, 8, 512], F16, name="xt")
                    xt_tiles[t] = xt
                xt = xt_tiles[t]
                fs = slice(4 * half, 4 * half + 4)
                eng.dma_start(out=xt[:, fs, :], in_=xT_d[:, t, fs, :])

            # DMA split: the first chunks and weights are split into halves
            # across the sync / gpsimd / scalar queues so their descriptor
            # streams drain in parallel (first matmuls gate on these).
            xt_fetch(0, nc.sync, 0)
            xt_fetch(0, nc.gpsimd, 1)
            nc.scalar.dma_start(out=wq_t[:, 0:4, :], in_=wq_d[:, 0:4, :])
            nc.scalar.dma_start(out=wq_t[:, 4:8, :], in_=wq_d[:, 4:8, :])
            xt_fetch(1, nc.sync, 0)
            xt_fetch(1, nc.gpsimd, 1)
            nc.scalar.dma_start(out=wk_t[:, 0:4, :], in_=wk_d[:, 0:4, :])
            nc.scalar.dma_start(out=wk_t[:, 4:8, :], in_=wk_d[:, 4:8, :])
            nc.scalar.dma_start(out=bq_t, in_=bq_d)
            nc.scalar.dma_start(out=bk_t, in_=bk_d)
            xt_fetch(2, nc.sync)
            nc.scalar.dma_start(out=wv_t, in_=wv_d)
            nc.scalar.dma_start(
                out=bva_t,
                in_=bass.AP(tensor=bva_d.tensor, offset=0,
                            ap=[[0, 128]] + list(bva_d.ap[-1:])),
            )
            for t in range(3, 8):
                xt_fetch(t, nc.gpsimd if t % 2 else nc.sync)
            nc.scalar.dma_start(out=wc_t, in_=wc_d)

            # v_t fixed columns: ones at 0 and 65, zeros at 130:200
            nc.vector.memset(dum_t, 0.0)
            nc.vector.memset(v_t[:, :, 0:1], 1.0)
            nc.vector.memset(v_t[:, :, 65:66], 1.0)
            nc.vector.memset(v_t[:, :, 130:200], 0.0)
            # selectors for the denominator-broadcast matmuls
            sel_t = consts.tile([128, 2, 128], F16, name="sel_t")
            nc.vector.memset(sel_t, 0.0)
            nc.vector.memset(sel_t[0:1, 0, 0:64], 1.0)
            nc.vector.memset(sel_t[0:1, 1, 64:128], 1.0)

            # ---- PE warmup: keep HAM busy while the first DMAs land ----
            psd = ps_c.tile([64, 64], F32, name="ps_c")
            for i in range(45):
                nc.tensor.matmul(psd, dum_t, dum_t,
                                 start=(i == 0), stop=(i == 44))

            # ---- projection filler units ----
            psq_pend = {}

            def proj_a(t, wt, key):
                ps = ps_c.tile([128, 512], F32, name="ps_c")
                psq_pend[key] = ps
                xt = xt_tiles[t]
                for f in range(4):
                    nc.tensor.matmul(ps, wt[:, f, :], xt[:, f, :],
                                     start=(f == 0), stop=False)

            def proj_b(t, wt, key, dst, bias):
                ps = psq_pend.pop(key)
                xt = xt_tiles[t]
                for f in range(4, 8):
                    nc.tensor.matmul(ps, wt[:, f, :], xt[:, f, :],
                                     start=False, stop=(f == 7))
                nc.vector.tensor_scalar_add(dst[:, ts(t, 512)], ps, bias)

            def proj_v_sub(t, sub):
                """one 128-token sub-chunk of V -> v_t[:, t*4+sub, :]"""
                xt = xt_tiles[t]
                c = t * 4 + sub
                psv = ps_c.tile([128, 512], F32, name="ps_c")
                for f in range(8):
                    nc.tensor.matmul(
                        psv[:, 0:128], xt[:, f, ts(sub, 128)], wv_t[:, f, :],
                        start=(f == 0), stop=(f == 7),
                    )
                nc.vector.tensor_add(v_t[:, c, 1:65], psv[:, 0:64], bva_t[:, 1:65])
                nc.vector.tensor_add(v_t[:, c, 66:130], psv[:, 64:128], bva_t[:, 66:130])

            def outproj_m(b, qc, m, tail=False):
                if tail:
                    pso = ps_st.tile([128, 512], F32, name="ps_st")
                else:
                    pso = ps_c.tile([128, 512], F32, name="ps_c")
                nc.tensor.matmul(
                    pso, wc_t[:, ts(m, 128)], oc_t[:, b, ts(qc, 512)],
                    start=True, stop=True,
                )
                so = outp.tile([128, 512], F16, name="so")
                if tail and m % 2:
                    nc.scalar.copy(so, pso)
                else:
                    nc.vector.tensor_copy(so, pso)
                eng = (nc.scalar if m % 2 else nc.sync) if tail else nc.gpsimd
                eng.dma_start(
                    out=outT_d[ts(m, 128), b * N + qc * 512: b * N + (qc + 1) * 512],
                    in_=so,
                )

            # ---- norm: phase a = PSUM drain + reciprocal (no psum
            # held); phase b (scheduled later) = PE broadcast + normalize ----
            rd_pend = {}

            def norm_a(b, qc, otps):
                qs = ts(qc, 512)
                rds = []
                for h in (0, 1):
                    # both halves: d at psum row 0, values rows 1:65
                    stg = stage.tile([65, 512], F16, name="stg")
                    nc.vector.tensor_copy(stg, otps[h][0:65, :])
                    nc.sync.dma_start(out=ot_t[h * 64:(h + 1) * 64, b, qs],
                                      in_=stg[1:65, :])
                    d1 = normp.tile([1, 512], F32, name="d0")
                    r1 = normp.tile([1, 512], F32, name="rd0")
                    r116 = normp.tile([1, 512], F16, name="rd0h")
                    nc.vector.tensor_copy(d1, otps[h][0:1, :])
                    nc.vector.reciprocal_approx_fast(out=r1, in_=d1)
                    nc.vector.tensor_copy(r116, r1)
                    rds.append(r116)
                rd_pend[(b, qc)] = rds

            def norm_b(b, qc):
                qs = ts(qc, 512)
                rds = rd_pend.pop((b, qc))
                bc_ps = ps_c.tile([128, 512], F32, name="ps_c")
                nc.tensor.matmul(bc_ps, sel_t[0:1, 0, :], rds[0],
                                 start=True, stop=False)
                nc.tensor.matmul(bc_ps, sel_t[0:1, 1, :], rds[1],
                                 start=False, stop=True)
                nc.vector.tensor_mul(oc_t[:, b, qs], ot_t[:, b, qs], bc_ps)

            # ---- static filler schedule -------------------------------------
            # unit vocab:  ('Qa',t) ('Qb',t) ('Ka',t) ('Kb',t) ('V',t,sub)
            #              ('OP',g)  -> one outproj m-chunk of group g (8 each)
            sched = {i: [] for i in range(128)}

            def put(slot, *u):
                sched[min(slot, 127)].append(u)

            # Emission-order rule: a filler that WRITES data must be
            # emitted at a slot strictly before the consumer's slot (the
            # tile framework cannot depend on writes emitted later).
            #   Kb_c  <= 4c-1   (scores(0,4c) reads kt chunk c at slot 4c)
            #   V_c,s <= 4c+s+3 (PV(0,kc) runs at slot kc+PV_LAG, after fillers
            #                    of the previous slot)
            put(0, 'V', 0, 0); put(0, 'Ka', 1)
            put(1, 'Kb', 1); put(1, 'V', 0, 1)
            put(2, 'V', 0, 2)
            put(3, 'V', 0, 3); put(3, 'Ka', 2)
            put(4, 'V', 1, 0)
            put(5, 'Kb', 2)
            put(6, 'V', 1, 1)
            put(7, 'V', 1, 2)
            put(8, 'V', 1, 3); put(8, 'Ka', 3)
            put(9, 'V', 2, 0); put(9, 'Kb', 3)
            put(10, 'V', 2, 1)
            put(11, 'V', 2, 2)
            put(12, 'V', 2, 3)
            put(13, 'V', 3, 0)
            put(14, 'V', 3, 1); put(14, 'Qa', 1)
            put(15, 'V', 3, 2); put(15, 'Qb', 1)
            put(16, 'V', 3, 3)
            put(18, 'Qa', 2); put(20, 'Qb', 2)
            put(22, 'Ka', 4); put(24, 'Kb', 4)
            put(26, 'V', 4, 0); put(28, 'V', 4, 1); put(30, 'V', 4, 2)
            put(32, 'Qa', 3); put(34, 'Qb', 3)
            put(36, 'V', 4, 3); put(38, 'V', 5, 0)
            put(40, 'Ka', 5); put(42, 'Kb', 5)
            put(44, 'V', 5, 1); put(46, 'V', 5, 2)
            put(48, 'Qa', 4); put(50, 'Qb', 4)
            put(52, 'V', 5, 3); put(54, 'V', 6, 0)
            put(56, 'Ka', 6); put(58, 'Kb', 6)
            put(60, 'V', 6, 1); put(62, 'V', 6, 2)
            put(64, 'Ka', 7); put(65, 'Kb', 7)
            put(66, 'V', 6, 3); put(68, 'V', 7, 0)
            put(70, 'V', 7, 1); put(72, 'V', 7, 2); put(74, 'V', 7, 3)
            put(77, 'Qa', 5); put(78, 'Qb', 5)
            put(82, 'Qa', 6); put(84, 'Qb', 6)
            put(98, 'Qa', 7); put(100, 'Qb', 7)
            # norm_b(g): after norm_a(g) at slot 16(g+1)+4, recips done
            for g in range(6):
                put(16 * (g + 1) + 8, 'NB', g)
            put(116, 'NB', 6)
            # outproj: group g's oc is written by norm_b(g)
            op_base = [40, 48, 80, 88, 96, 106, 117]
            for g in range(7):
                stride = 1 if g == 6 else 2
                for m in range(8):
                    put(op_base[g] + stride * m, 'OP', g)
            op_done = [0] * 8

            def run_unit(u):
                kind = u[0]
                if kind == 'Qa':
                    proj_a(u[1], wq_t, ('q', u[1]))
                elif kind == 'Qb':
                    proj_b(u[1], wq_t, ('q', u[1]), qt_t, bq_t)
                elif kind == 'Ka':
                    proj_a(u[1], wk_t, ('k', u[1]))
                elif kind == 'Kb':
                    proj_b(u[1], wk_t, ('k', u[1]), kt_t, bk_t)
                elif kind == 'V':
                    proj_v_sub(u[1], u[2])
                elif kind == 'NB':
                    norm_b(u[1] // 4, u[1] % 4)
                elif kind == 'OP':
                    g = u[1]
                    outproj_m(g // 4, g % 4, op_done[g])
                    op_done[g] += 1

            # ---- head: QK projection of chunk 0 ----
            proj_a(0, wq_t, ('q', 0)); proj_b(0, wq_t, ('q', 0), qt_t, bq_t)
            proj_a(0, wk_t, ('k', 0)); proj_b(0, wk_t, ('k', 0), kt_t, bk_t)

            # ---- flat attention pipeline over 128 (group, kc) slots ----
            PV_LAG = 4
            pend = []               # (b, qc, kc, pt) awaiting PV
            cur_otps = None         # PSUM accumulators of the PV-active group
            prev_group = None       # (b, qc, otps) awaiting norm

            def do_pv(pb, pqc, pkc, ppt):
                nonlocal cur_otps, prev_group
                if pkc == 0:
                    cur_otps = [ps_ot.tile([128, 512], F32, name=f"ps_ot{h}")
                                for h in (0, 1)]
                for h in (0, 1):
                    nc.tensor.matmul(
                        cur_otps[h],
                        v_t[:, pb * 16 + pkc, h * 65:h * 65 + 128],
                        ppt[:, ts(h, 512)],
                        start=(pkc == 0), stop=(pkc == 15),
                    )
                if pkc == 15:
                    prev_group = (pb, pqc, cur_otps)

            for i in range(128):
                g, kc = i // 16, i % 16
                b, qc = g // 4, g % 4
                qoff = b * N + qc * 512
                st = ps_st.tile([128, 1024], F32, name="ps_st")
                pt = ptp.tile([128, 1024], F16, name="pt")
                koff = b * N + kc * 128
                for h in (0, 1):
                    lo = h * 64
                    nc.tensor.matmul(
                        st[:, ts(h, 512)],
                        kt_t[lo:lo + 64, koff:koff + 128],
                        qt_t[lo:lo + 64, qoff:qoff + 512],
                        start=True, stop=True,
                    )
                nc.scalar.activation(
                    pt, st, mybir.ActivationFunctionType.Exp, scale=SCALE,
                )
                # norm(g-1) before this slot's PV so the accumulator pool's
                # re-allocation (at pkc==0) happens after all of its reads
                if prev_group is not None:
                    ng = prev_group
                    prev_group = None
                    norm_a(ng[0], ng[1], ng[2])
                if len(pend) >= PV_LAG:
                    do_pv(*pend.pop(0))
                pend.append((b, qc, kc, pt))
                for u in sched[i]:
                    run_unit(u)

            # ---- epilogue ----
            while pend:
                if prev_group is not None:
                    ng = prev_group
                    prev_group = None
                    norm_a(ng[0], ng[1], ng[2])
                do_pv(*pend.pop(0))
            # keep the PE busy (HAM warm) while the final norm chain runs
            psd2 = ps_c.tile([64, 64], F32, name="ps_c")
            for i in range(58):
                nc.tensor.matmul(psd2, dum_t, dum_t,
                                 start=(i == 0), stop=(i == 57))
            norm_a(1, 3, cur_otps)
            for g in range(7):
                for m in range(8 - op_done[g]):
                    run_unit(('OP', g))
            norm_b(1, 3)
            for m in range(8):
                outproj_m(1, 3, m, tail=True)

    nc.compile()
    return nc


def kernel(x, Wq, bq, Wk, bk, Wv, bv, Wp, bp,
           lambda_q1, lambda_k1, lambda_q2, lambda_k2):
    x = np.asarray(x, dtype=np.float32)
    Wq, Wk, Wv, Wp = [np.asarray(w, dtype=np.float32) for w in (Wq, Wk, Wv, Wp)]
    bq, bk, bv, bp = [np.asarray(v, dtype=np.float32) for v in (bq, bk, bv, bp)]

    l1 = np.exp(np.minimum(
        (np.asarray(lambda_q1, np.float32) * np.asarray(lambda_k1, np.float32))
        .sum((-1, -2)), 5.0))
    l2 = np.exp(np.minimum(
        (np.asarray(lambda_q2, np.float32) * np.asarray(lambda_k2, np.float32))
        .sum((-1, -2)), 5.0))
    lv = np.float32((l1 - l2 + np.float32(LAMBDA_INIT)).mean())

    xT = x.reshape(T, EMBED).T.astype(np.float16)
    xT = np.ascontiguousarray(xT.reshape(8, 128, 8, 512).transpose(1, 2, 0, 3))

    if _compiled[0] is None:
        _compiled[0] = _build()
    nc = _compiled[0]

    in_maps = []
    for p in range(NCORES):
        r1 = slice(p * HD, (p + 1) * HD)          # head p rows/cols
        r2 = slice((8 + p) * HD, (9 + p) * HD)    # head p+8 rows/cols
        wq_p = np.concatenate([Wq[r1], Wq[r2]], 0).T      # [1024, 128]
        wk_p = np.concatenate([Wk[r1], Wk[r2]], 0).T
        wv_p = np.concatenate([Wv[r1], Wv[r2]], 0).T
        wpt1 = Wp[:, r1].T                                 # [64, 1024]
        wpt2 = Wp[:, r2].T
        wcomb = np.concatenate([wpt1, wpt2 - lv * wpt1], 0)  # [128, 1024]
        bva = np.concatenate(
            [[1.0], bv[r1], [1.0], bv[r2]]).astype(np.float32)[None, :]
        in_maps.append({
            "xT": xT,
            "wq": np.ascontiguousarray(
                wq_p.reshape(8, 128, 128).transpose(1, 0, 2).astype(np.float16)),
            "wk": np.ascontiguousarray(
                wk_p.reshape(8, 128, 128).transpose(1, 0, 2).astype(np.float16)),
            "wv": np.ascontiguousarray(
                wv_p.reshape(8, 128, 128).transpose(1, 0, 2).astype(np.float16)),
            "wcomb": np.ascontiguousarray(wcomb.astype(np.float16)),
            "bq": np.concatenate([bq[r1], bq[r2]])[:, None].copy(),
            "bk": np.concatenate([bk[r1], bk[r2]])[:, None].copy(),
            "bvaug": np.ascontiguousarray(bva),
        })

    res = run_bass_kernel_spmd(
        nc, in_maps, core_ids=list(range(NCORES)), trace=TRACE,
    )
    LAST_RESULT[0] = res

    outT = res.results[0]["outT"].astype(np.float64)
    for c in range(1, NCORES):
        outT += res.results[c]["outT"]
    out = outT.T.reshape(B, N, EMBED).astype(np.float32) + bp[None, None, :]
    return out


# revision 19
# speedup vs baseline: 1.0025x; 1.0025x over previous
"""Differential multi-head attention on 8 Trainium2 NeuronCores.

Sharding: core p owns head pair (p, p+8) for both batches (tensor parallel
over the 8 differential head pairs). lambda scalars are folded into the
output-projection weights on the host. Host sums the 8 partial outputs.

v2 schedule: the ACT engine's exp stream (128 x [128,1024] activations,
~142us) is the hard floor; everything else is laid out to keep it gapless:
 - flat (group, kc) software pipeline: scores(i) -> exp(i) -> PV(i-1)
 - projections split into ~0.9us units, EDF-placed as PE filler inside the
   attention loop (b1's K/V prefetched during b0's ACT-bound groups)
 - softmax denominators: DVE reciprocal straight from PSUM row 64, then
   GpSimd partition_broadcast (no DRAM round trip)
 - all output-projection work deferred to b1 groups + epilogue
 - PE warmup dummies during the initial DMA wait (HAM un-throttle)
"""
import numpy as np

import concourse.bacc as bacc
import concourse.bass as bass
import concourse.tile as tile
import concourse.mybir as mybir
from concourse.bass_utils import run_bass_kernel_spmd

F32 = mybir.dt.float32
F16 = mybir.dt.float16

EMBED = 1024
H2 = 8
HD = 64
B = 2
N = 2048
T = B * N  # 4096
NCORES = 8
LAMBDA_INIT = 0.8
SCALE = HD ** -0.5

TRACE = False
LAST_RESULT = [None]

_compiled = [None]


def ts(i, size):
    return slice(i * size, (i + 1) * size)


def _build():
    nc = bacc.Bacc("TRN2", target_bir_lowering=False, debug=False, num_devices=NCORES)

    xT_d = nc.dram_tensor("xT", [128, 8, 8, 512], F16, kind="ExternalInput").ap()
    wq_d = nc.dram_tensor("wq", [128, 8, 128], F16, kind="ExternalInput").ap()
    wk_d = nc.dram_tensor("wk", [128, 8, 128], F16, kind="ExternalInput").ap()
    wv_d = nc.dram_tensor("wv", [128, 8, 128], F16, kind="ExternalInput").ap()
    wc_d = nc.dram_tensor("wcomb", [128, 1024], F16, kind="ExternalInput").ap()
    bq_d = nc.dram_tensor("bq", [128, 1], F32, kind="ExternalInput").ap()
    bk_d = nc.dram_tensor("bk", [128, 1], F32, kind="ExternalInput").ap()
    bva_d = nc.dram_tensor("bvaug", [1, 130], F32, kind="ExternalInput").ap()
    outT_d = nc.dram_tensor("outT", [EMBED, T], F16, kind="ExternalOutput").ap()

    with tile.TileContext(nc) as tc:
        with (
            tc.tile_pool(name="consts", bufs=1) as consts,
            tc.tile_pool(name="xp", bufs=8) as xp,
            tc.tile_pool(name="qkv", bufs=1) as qkv,
            tc.tile_pool(name="ptp", bufs=8) as ptp,
            tc.tile_pool(name="stage", bufs=3) as stage,
            tc.tile_pool(name="normp", bufs=2) as normp,
            tc.tile_pool(name="outp", bufs=4) as outp,
            tc.tile_pool(name="ps_st", bufs=2, space="PSUM") as ps_st,
            tc.tile_pool(name="ps_ot", bufs=1, space="PSUM") as ps_ot,
            tc.tile_pool(name="ps_c", bufs=2, space="PSUM") as ps_c,
        ):
            # ---- constant / input tiles ----
            wq_t = consts.tile([128, 8, 128], F16, name="wq_t")
            wk_t = consts.tile([128, 8, 128], F16, name="wk_t")
            wv_t = consts.tile([128, 8, 128], F16, name="wv_t")
            wc_t = consts.tile([128, 1024], F16, name="wc_t")
            bq_t = consts.tile([128, 1], F32, name="bq_t")
            bk_t = consts.tile([128, 1], F32, name="bk_t")
            bva_t = consts.tile([128, 130], F32, name="bva_t")
            dum_t = consts.tile([128, 64], F16, name="dum_t")

            qt_t = qkv.tile([128, T], F16, name="qt_t")
            kt_t = qkv.tile([128, T], F16, name="kt_t")
            v_t = qkv.tile([128, 32, 200], F16, name="v_t")
            ot_t = qkv.tile([128, B, N], F16, name="ot_t")
            oc_t = qkv.tile([128, B, N], F16, name="oc_t")

            xt_tiles = {}

            def xt_fetch(t, eng=None, half=None):
                if half is None:
                    xt = xp.tile([128, 8, 512], F16, name="xt")
                    eng.dma_start(out=xt, in_=xT_d[:, t, :, :])
                    xt_tiles[t] = xt
                    return
                if half == 0:
                    xt = xp.tile([128, 8, 512], F16, name="xt")
                    xt_tiles[t] = xt
                xt = xt_tiles[t]
                fs = slice(4 * half, 4 * half + 4)
                eng.dma_start(out=xt[:, fs, :], in_=xT_d[:, t, fs, :])

            # DMA split: the first chunks and weights are split into halves
            # across the sync / gpsimd / scalar queues so their descriptor
            # streams drain in parallel (first matmuls gate on these).
            xt_fetch(0, nc.sync, 0)
            xt_fetch(0, nc.gpsimd, 1)
            nc.scalar.dma_start(out=wq_t[:, 0:4, :], in_=wq_d[:, 0:4, :])
            nc.scalar.dma_start(out=wq_t[:, 4:8, :], in_=wq_d[:, 4:8, :])
            xt_fetch(1, nc.sync, 0)
            xt_fetch(1, nc.gpsimd, 1)
            nc.scalar.dma_start(out=wk_t[:, 0:4, :], in_=wk_d[:, 0:4, :])
            nc.scalar.dma_start(out=wk_t[:, 4:8, :], in_=wk_d[:, 4:8, :])
            nc.scalar.dma_start(out=bq_t, in_=bq_d)
            nc.scalar.dma_start(out=bk_t, in_=bk_d)
            xt_fetch(2, nc.sync)
            nc.scalar.dma_start(out=wv_t, in_=wv_d)
            nc.scalar.dma_start(
                out=bva_t,
                in_=bass.AP(tensor=bva_d.tensor, offset=0,
                            ap=[[0, 128]] + list(bva_d.ap[-1:])),
            )
            for t in range(3, 8):
                xt_fetch(t, nc.gpsimd if t % 2 else nc.sync)
            nc.scalar.dma_start(out=wc_t, in_=wc_d)

            # v_t fixed columns: ones at 0 and 65, zeros at 130:200
            nc.vector.memset(dum_t, 0.0)
            nc.vector.memset(v_t[:, :, 0:1], 1.0)
            nc.vector.memset(v_t[:, :, 65:66], 1.0)
            nc.vector.memset(v_t[:, :, 130:200], 0.0)
            # selectors for the denominator-broadcast matmuls
            sel_t = consts.tile([128, 2, 128], F16, name="sel_t")
            nc.vector.memset(sel_t, 0.0)
            nc.vector.memset(sel_t[0:1, 0, 0:64], 1.0)
            nc.vector.memset(sel_t[0:1, 1, 64:128], 1.0)

            # ---- PE warmup: keep HAM busy while the first DMAs land ----
            psd = ps_c.tile([64, 64], F32, name="ps_c")
            for i in range(45):
                nc.tensor.matmul(psd, dum_t, dum_t,
                                 start=(i == 0), stop=(i == 44))

            # ---- projection filler units ----
            psq_pend = {}

            def proj_a(t, wt, key):
                ps = ps_c.tile([128, 512], F32, name="ps_c")
                psq_pend[key] = ps
                xt = xt_tiles[t]
                for f in range(4):
                    nc.tensor.matmul(ps, wt[:, f, :], xt[:, f, :],
                                     start=(f == 0), stop=False)

            def proj_b(t, wt, key, dst, bias):
                ps = psq_pend.pop(key)
                xt = xt_tiles[t]
                for f in range(4, 8):
                    nc.tensor.matmul(ps, wt[:, f, :], xt[:, f, :],
                                     start=False, stop=(f == 7))
                nc.vector.tensor_scalar_add(dst[:, ts(t, 512)], ps, bias)

            def proj_v_sub(t, sub):
                """one 128-token sub-chunk of V -> v_t[:, t*4+sub, :]"""
                xt = xt_tiles[t]
                c = t * 4 + sub
                psv = ps_c.tile([128, 512], F32, name="ps_c")
                for f in range(8):
                    nc.tensor.matmul(
                        psv[:, 0:128], xt[:, f, ts(sub, 128)], wv_t[:, f, :],
                        start=(f == 0), stop=(f == 7),
                    )
                nc.vector.tensor_add(v_t[:, c, 1:65], psv[:, 0:64], bva_t[:, 1:65])
                nc.vector.tensor_add(v_t[:, c, 66:130], psv[:, 64:128], bva_t[:, 66:130])

            def outproj_m(b, qc, m, tail=False):
                if tail:
                    pso = ps_st.tile([128, 512], F32, name="ps_st")
                else:
                    pso = ps_c.tile([128, 512], F32, name="ps_c")
                nc.tensor.matmul(
                    pso, wc_t[:, ts(m, 128)], oc_t[:, b, ts(qc, 512)],
                    start=True, stop=True,
                )
                so = outp.tile([128, 512], F16, name="so")
                if tail and m % 2:
                    nc.scalar.copy(so, pso)
                else:
                    nc.vector.tensor_copy(so, pso)
                eng = (nc.scalar if m % 2 else nc.sync) if tail else nc.gpsimd
                eng.dma_start(
                    out=outT_d[ts(m, 128), b * N + qc * 512: b * N + (qc + 1) * 512],
                    in_=so,
                )

            # ---- norm: phase a = PSUM drain + reciprocal (no psum
            # held); phase b (scheduled later) = PE broadcast + normalize ----
            rd_pend = {}

            def norm_a(b, qc, otps):
                qs = ts(qc, 512)
                rds = []
                for h in (0, 1):
                    # both halves: d at psum row 0, values rows 1:65
                    stg = stage.tile([65, 512], F16, name="stg")
                    nc.vector.tensor_copy(stg, otps[h][0:65, :])
                    nc.sync.dma_start(out=ot_t[h * 64:(h + 1) * 64, b, qs],
                                      in_=stg[1:65, :])
                    d1 = normp.tile([1, 512], F32, name="d0")
                    r1 = normp.tile([1, 512], F32, name="rd0")
                    r116 = normp.tile([1, 512], F16, name="rd0h")
                    nc.vector.tensor_copy(d1, otps[h][0:1, :])
                    nc.vector.reciprocal_approx_fast(out=r1, in_=d1)
                    nc.vector.tensor_copy(r116, r1)
                    rds.append(r116)
                rd_pend[(b, qc)] = rds

            def norm_b(b, qc):
                qs = ts(qc, 512)
                rds = rd_pend.pop((b, qc))
                bc_ps = ps_c.tile([128, 512], F32, name="ps_c")
                nc.tensor.matmul(bc_ps, sel_t[0:1, 0, :], rds[0],
                                 start=True, stop=False)
                nc.tensor.matmul(bc_ps, sel_t[0:1, 1, :], rds[1],
                                 start=False, stop=True)
                nc.vector.tensor_mul(oc_t[:, b, qs], ot_t[:, b, qs], bc_ps)

            # ---- static filler schedule -------------------------------------
            # unit vocab:  ('Qa',t) ('Qb',t) ('Ka',t) ('Kb',t) ('V',t,sub)
            #              ('OP',g)  -> one outproj m-chunk of group g (8 each)
            sched = {i: [] for i in range(128)}

            def put(slot, *u):
                sched[min(slot, 127)].append(u)

            # Emission-order rule: a filler that WRITES data must be
            # emitted at a slot strictly before the consumer's slot (the
            # tile framework cannot depend on writes emitted later).
            #   Kb_c  <= 4c-1   (scores(0,4c) reads kt chunk c at slot 4c)
            #   V_c,s <= 4c+s+3 (PV(0,kc) runs at slot kc+PV_LAG, after fillers
            #                    of the previous slot)
            put(0, 'V', 0, 0); put(0, 'Ka', 1)
            put(1, 'Kb', 1); put(1, 'V', 0, 1)
            put(2, 'V', 0, 2)
            put(3, 'V', 0, 3); put(3, 'Ka', 2)
            put(4, 'V', 1, 0)
            put(5, 'Kb', 2)
            put(6, 'V', 1, 1)
            put(7, 'V', 1, 2)
            put(8, 'V', 1, 3); put(8, 'Ka', 3)
            put(9, 'V', 2, 0); put(9, 'Kb', 3)
            put(10, 'V', 2, 1)
            put(11, 'V', 2, 2)
            put(12, 'V', 2, 3)
            put(13, 'V', 3, 0)
            put(14, 'V', 3, 1); put(14, 'Qa', 1)
            put(15, 'V', 3, 2); put(15, 'Qb', 1)
            put(16, 'V', 3, 3)
            put(18, 'Qa', 2); put(20, 'Qb', 2)
            put(22, 'Ka', 4); put(24, 'Kb', 4)
            put(26, 'V', 4, 0); put(28, 'V', 4, 1); put(30, 'V', 4, 2)
            put(32, 'Qa', 3); put(34, 'Qb', 3)
            put(36, 'V', 4, 3); put(38, 'V', 5, 0)
            put(40, 'Ka', 5); put(42, 'Kb', 5)
            put(44, 'V', 5, 1); put(46, 'V', 5, 2)
            put(48, 'Qa', 4); put(50, 'Qb', 4)
            put(52, 'V', 5, 3); put(54, 'V', 6, 0)
            put(56, 'Ka', 6); put(58, 'Kb', 6)
            put(60, 'V', 6, 1); put(62, 'V', 6, 2)
            put(64, 'Ka', 7); put(65, 'Kb', 7)
            put(66, 'V', 6, 3); put(68, 'V', 7, 0)
            put(70, 'V', 7, 1); put(72, 'V', 7, 2); put(74, 'V', 7, 3)
            put(77, 'Qa', 5); put(78, 'Qb', 5)
            put(82, 'Qa', 6); put(84, 'Qb', 6)
            put(98, 'Qa', 7); put(100, 'Qb', 7)
            # norm_b(g): after norm_a(g) at slot 16(g+1)+4, recips done
            for g in range(6):
                put(16 * (g + 1) + 8, 'NB', g)
            put(116, 'NB', 6)
            # outproj: group g's oc is written by norm_b(g)
            op_base = [40, 48, 80, 88, 96, 106, 117]
            for g in range(7):
                stride = 1 if g == 6 else 2
                for m in range(8):
                    put(op_base[g] + stride * m, 'OP', g)
            op_done = [0] * 8

            def run_unit(u):
                kind = u[0]
                if kind == 'Qa':
                    proj_a(u[1], wq_t, ('q', u[1]))
                elif kind == 'Qb':
                    proj_b(u[1], wq_t, ('q', u[1]), qt_t, bq_t)
                elif kind == 'Ka':
                    proj_a(u[1], wk_t, ('k', u[1]))
                elif kind == 'Kb':
                    proj_b(u[1], wk_t, ('k', u[1]), kt_t, bk_t)
                elif kind == 'V':
                    proj_v_sub(u[1], u[2])
                elif kind == 'NB':
                    norm_b(u[1] // 4, u[1] % 4)
                elif kind == 'OP':
                    g = u[1]
                    outproj_m(g // 4, g % 4, op_done[g])
                    op_done[g] += 1

            # ---- head: QK projection of chunk 0 ----
            proj_a(0, wq_t, ('q', 0)); proj_b(0, wq_t, ('q', 0), qt_t, bq_t)
            proj_a(0, wk_t, ('k', 0)); proj_b(0, wk_t, ('k', 0), kt_t, bk_t)

            # ---- flat attention pipeline over 128 (group, kc) slots ----
            PV_LAG = 4
            pend = []               # (b, qc, kc, pt) awaiting PV
            cur_otps = None         # PSUM accumulators of the PV-active group
            prev_group = None       # (b, qc, otps) awaiting norm

            def do_pv(pb, pqc, pkc, ppt):
                nonlocal cur_otps, prev_group
                if pkc == 0:
                    cur_otps = [ps_ot.tile([128, 512], F32, name=f"ps_ot{h}")
                                for h in (0, 1)]
                for h in (0, 1):
                    nc.tensor.matmul(
                        cur_otps[h],
                        v_t[:, pb * 16 + pkc, h * 65:h * 65 + 128],
                        ppt[:, ts(h, 512)],
                        start=(pkc == 0), stop=(pkc == 15),
                    )
                if pkc == 15:
                    prev_group = (pb, pqc, cur_otps)

            for i in range(128):
                g, kc = i // 16, i % 16
                b, qc = g // 4, g % 4
                qoff = b * N + qc * 512
                st = ps_st.tile([128, 1024], F32, name="ps_st")
                pt = ptp.tile([128, 1024], F16, name="pt")
                koff = b * N + kc * 128
                for h in (0, 1):
                    lo = h * 64
                    nc.tensor.matmul(
                        st[:, ts(h, 512)],
                        kt_t[lo:lo + 64, koff:koff + 128],
                        qt_t[lo:lo + 64, qoff:qoff + 512],
                        start=True, stop=True,
                    )
                nc.scalar.activation(
                    pt, st, mybir.ActivationFunctionType.Exp, scale=SCALE,
                )
                # norm(g-1) before this slot's PV so the accumulator pool's
                # re-allocation (at pkc==0) happens after all of its reads
                if prev_group is not None:
                    ng = prev_group
                    prev_group = None
                    norm_a(ng[0], ng[1], ng[2])
                if len(pend) >= PV_LAG:
                    do_pv(*pend.pop(0))
                pend.append((b, qc, kc, pt))
                for u in sched[i]:
                    run_unit(u)

            # ---- epilogue ----
            while pend:
                if prev_group is not None:
                    ng = prev_group
                    prev_group = None
                    norm_a(ng[0], ng[1], ng[2])
                do_pv(*pend.pop(0))
            # keep the PE busy (HAM warm) while the final norm chain runs
            psd2 = ps_c.tile([64, 64], F32, name="ps_c")
            for i in range(58):
                nc.tensor.matmul(psd2, dum_t, dum_t,
                                 start=(i == 0), stop=(i == 57))
            norm_a(1, 3, cur_otps)
            for g in range(7):
                for m in range(8 - op_done[g]):
                    run_unit(('OP', g))
            norm_b(1, 3)
            for m in range(8):
                outproj_m(1, 3, m, tail=True)

    nc.compile()
    return nc


def kernel(x, Wq, bq, Wk, bk, Wv, bv, Wp, bp,
           lambda_q1, lambda_k1, lambda_q2, lambda_k2):
    x = np.asarray(x, dtype=np.float32)
    Wq, Wk, Wv, Wp = [np.asarray(w, dtype=np.float32) for w in (Wq, Wk, Wv, Wp)]
    bq, bk, bv, bp = [np.asarray(v, dtype=np.float32) for v in (bq, bk, bv, bp)]

    l1 = np.exp(np.minimum(
        (np.asarray(lambda_q1, np.float32) * np.asarray(lambda_k1, np.float32))
        .sum((-1, -2)), 5.0))
    l2 = np.exp(np.minimum(
        (np.asarray(lambda_q2, np.float32) * np.asarray(lambda_k2, np.float32))
        .sum((-1, -2)), 5.0))
    lv = np.float32((l1 - l2 + np.float32(LAMBDA_INIT)).mean())

    xT = x.reshape(T, EMBED).T.astype(np.float16)
    xT = np.ascontiguousarray(xT.reshape(8, 128, 8, 512).transpose(1, 2, 0, 3))

    if _compiled[0] is None:
        _compiled[0] = _build()
    nc = _compiled[0]

    in_maps = []
    for p in range(NCORES):
        r1 = slice(p * HD, (p + 1) * HD)          # head p rows/cols
        r2 = slice((8 + p) * HD, (9 + p) * HD)    # head p+8 rows/cols
        wq_p = np.concatenate([Wq[r1], Wq[r2]], 0).T      # [1024, 128]
        wk_p = np.concatenate([Wk[r1], Wk[r2]], 0).T
        wv_p = np.concatenate([Wv[r1], Wv[r2]], 0).T
        wpt1 = Wp[:, r1].T                                 # [64, 1024]
        wpt2 = Wp[:, r2].T
        wcomb = np.concatenate([wpt1, wpt2 - lv * wpt1], 0)  # [128, 1024]
        bva = np.concatenate(
            [[1.0], bv[r1], [1.0], bv[r2]]).astype(np.float32)[None, :]
        in_maps.append({
            "xT": xT,
            "wq": np.ascontiguousarray(
                wq_p.reshape(8, 128, 128).transpose(1, 0, 2).astype(np.float16)),
            "wk": np.ascontiguousarray(
                wk_p.reshape(8, 128, 128).transpose(1, 0, 2).astype(np.float16)),
            "wv": np.ascontiguousarray(
                wv_p.reshape(8, 128, 128).transpose(1, 0, 2).astype(np.float16)),
            "wcomb": np.ascontiguousarray(wcomb.astype(np.float16)),
            "bq": np.concatenate([bq[r1], bq[r2]])[:, None].copy(),
            "bk": np.concatenate([bk[r1], bk[r2]])[:, None].copy(),
            "bvaug": np.ascontiguousarray(bva),
        })

    res = run_bass_kernel_spmd(
        nc, in_maps, core_ids=list(range(NCORES)), trace=TRACE,
    )
    LAST_RESULT[0] = res

    outT = res.results[0]["outT"].astype(np.float64)
    for c in range(1, NCORES):
        outT += res.results[c]["outT"]
    out = outT.T.reshape(B, N, EMBED).astype(np.float32) + bp[None, None, :]
    return out


# revision 20
# speedup vs baseline: 1.0474x; 1.0448x over previous
"""Differential multi-head attention on 8 Trainium2 NeuronCores.

Sharding: core p owns head pair (p, p+8) for both batches (tensor parallel
over the 8 differential head pairs). lambda scalars are folded into the
output-projection weights on the host. Host sums the 8 partial outputs.

v2 schedule: the ACT engine's exp stream (128 x [128,1024] activations,
~142us) is the hard floor; everything else is laid out to keep it gapless:
 - flat (group, kc) software pipeline: scores(i) -> exp(i) -> PV(i-1)
 - projections split into ~0.9us units, EDF-placed as PE filler inside the
   attention loop (b1's K/V prefetched during b0's ACT-bound groups)
 - softmax denominators: DVE reciprocal straight from PSUM row 64, then
   GpSimd partition_broadcast (no DRAM round trip)
 - all output-projection work deferred to b1 groups + epilogue
 - PE warmup dummies during the initial DMA wait (HAM un-throttle)
"""
import numpy as np

import concourse.bacc as bacc
import concourse.bass as bass
import concourse.tile as tile
import concourse.mybir as mybir
from concourse.bass_utils import run_bass_kernel_spmd

F32 = mybir.dt.float32
F16 = mybir.dt.float16

EMBED = 1024
H2 = 8
HD = 64
B = 2
N = 2048
T = B * N  # 4096
NCORES = 8
LAMBDA_INIT = 0.8
SCALE = HD ** -0.5

TRACE = False
LAST_RESULT = [None]

_compiled = [None]


def ts(i, size):
    return slice(i * size, (i + 1) * size)


def _build():
    nc = bacc.Bacc("TRN2", target_bir_lowering=False, debug=False, num_devices=NCORES)

    xT_d = nc.dram_tensor("xT", [128, 8, 8, 512], F16, kind="ExternalInput").ap()
    wqk_d = nc.dram_tensor("wqk", [128, 16, 128], F16, kind="ExternalInput").ap()
    wv_d = nc.dram_tensor("wv", [128, 8, 128], F16, kind="ExternalInput").ap()
    wc_d = nc.dram_tensor("wcomb", [128, 1024], F16, kind="ExternalInput").ap()
    bqk_d = nc.dram_tensor("bqk", [128, 2], F32, kind="ExternalInput").ap()
    bva_d = nc.dram_tensor("bvaug", [1, 130], F32, kind="ExternalInput").ap()
    outT_d = nc.dram_tensor("outT", [EMBED, T], F16, kind="ExternalOutput").ap()

    with tile.TileContext(nc) as tc:
        with (
            tc.tile_pool(name="consts", bufs=1) as consts,
            tc.tile_pool(name="xp", bufs=8) as xp,
            tc.tile_pool(name="qkv", bufs=1) as qkv,
            tc.tile_pool(name="ptp", bufs=8) as ptp,
            tc.tile_pool(name="stage", bufs=3) as stage,
            tc.tile_pool(name="normp", bufs=2) as normp,
            tc.tile_pool(name="outp", bufs=4) as outp,
            tc.tile_pool(name="ps_st", bufs=2, space="PSUM") as ps_st,
            tc.tile_pool(name="ps_ot", bufs=1, space="PSUM") as ps_ot,
            tc.tile_pool(name="ps_c", bufs=2, space="PSUM") as ps_c,
        ):
            # ---- constant / input tiles ----
            wqk_t = consts.tile([128, 16, 128], F16, name="wqk_t")
            wq_t = wqk_t[:, 0:8, :]
            wk_t = wqk_t[:, 8:16, :]
            wv_t = consts.tile([128, 8, 128], F16, name="wv_t")
            wc_t = consts.tile([128, 1024], F16, name="wc_t")
            bqk_t = consts.tile([128, 2], F32, name="bqk_t")
            bq_t = bqk_t[:, 0:1]
            bk_t = bqk_t[:, 1:2]
            bva_t = consts.tile([128, 130], F32, name="bva_t")
            dum_t = consts.tile([128, 64], F16, name="dum_t")

            qt_t = qkv.tile([128, T], F16, name="qt_t")
            kt_t = qkv.tile([128, T], F16, name="kt_t")
            v_t = qkv.tile([128, 32, 200], F16, name="v_t")
            ot_t = qkv.tile([128, B, N], F16, name="ot_t")
            oc_t = qkv.tile([128, B, N], F16, name="oc_t")

            xt_tiles = {}

            def xt_fetch(t, eng):
                xt = xp.tile([128, 8, 512], F16, name="xt")
                eng.dma_start(out=xt, in_=xT_d[:, t, :, :])
                xt_tiles[t] = xt

            # Early DMAs are ~5us latency per queue item regardless of size,
            # so the critical path (xt0 + qk weights + biases) is exactly one
            # item on each of the three DMA-capable queues.
            xt_fetch(0, nc.sync)
            nc.scalar.dma_start(out=wqk_t, in_=wqk_d)
            nc.gpsimd.dma_start(out=bqk_t, in_=bqk_d)
            xt_fetch(1, nc.sync)
            nc.gpsimd.dma_start(out=wv_t, in_=wv_d)
            nc.scalar.dma_start(
                out=bva_t,
                in_=bass.AP(tensor=bva_d.tensor, offset=0,
                            ap=[[0, 128]] + list(bva_d.ap[-1:])),
            )
            xt_fetch(2, nc.gpsimd)
            for t in range(3, 8):
                xt_fetch(t, nc.gpsimd if t % 2 else nc.sync)
            nc.scalar.dma_start(out=wc_t, in_=wc_d)

            # v_t fixed columns: ones at 0 and 65, zeros at 130:200
            nc.vector.memset(dum_t, 0.0)
            nc.vector.memset(v_t[:, :, 0:1], 1.0)
            nc.vector.memset(v_t[:, :, 65:66], 1.0)
            nc.vector.memset(v_t[:, :, 130:200], 0.0)
            # selectors for the denominator-broadcast matmuls
            sel_t = consts.tile([128, 2, 128], F16, name="sel_t")
            nc.vector.memset(sel_t, 0.0)
            nc.vector.memset(sel_t[0:1, 0, 0:64], 1.0)
            nc.vector.memset(sel_t[0:1, 1, 64:128], 1.0)

            # ---- PE warmup: keep HAM busy while the first DMAs land ----
            psd = ps_c.tile([64, 64], F32, name="ps_c")
            for i in range(45):
                nc.tensor.matmul(psd, dum_t, dum_t,
                                 start=(i == 0), stop=(i == 44))

            # ---- projection filler units ----
            psq_pend = {}

            def proj_a(t, wt, key):
                ps = ps_c.tile([128, 512], F32, name="ps_c")
                psq_pend[key] = ps
                xt = xt_tiles[t]
                for f in range(4):
                    nc.tensor.matmul(ps, wt[:, f, :], xt[:, f, :],
                                     start=(f == 0), stop=False)

            def proj_b(t, wt, key, dst, bias):
                ps = psq_pend.pop(key)
                xt = xt_tiles[t]
                for f in range(4, 8):
                    nc.tensor.matmul(ps, wt[:, f, :], xt[:, f, :],
                                     start=False, stop=(f == 7))
                nc.vector.tensor_scalar_add(dst[:, ts(t, 512)], ps, bias)

            def proj_v_sub(t, sub):
                """one 128-token sub-chunk of V -> v_t[:, t*4+sub, :]"""
                xt = xt_tiles[t]
                c = t * 4 + sub
                psv = ps_c.tile([128, 512], F32, name="ps_c")
                for f in range(8):
                    nc.tensor.matmul(
                        psv[:, 0:128], xt[:, f, ts(sub, 128)], wv_t[:, f, :],
                        start=(f == 0), stop=(f == 7),
                    )
                nc.vector.tensor_add(v_t[:, c, 1:65], psv[:, 0:64], bva_t[:, 1:65])
                nc.vector.tensor_add(v_t[:, c, 66:130], psv[:, 64:128], bva_t[:, 66:130])

            def outproj_m(b, qc, m, tail=False):
                if tail:
                    pso = ps_st.tile([128, 512], F32, name="ps_st")
                else:
                    pso = ps_c.tile([128, 512], F32, name="ps_c")
                nc.tensor.matmul(
                    pso, wc_t[:, ts(m, 128)], oc_t[:, b, ts(qc, 512)],
                    start=True, stop=True,
                )
                so = outp.tile([128, 512], F16, name="so")
                if tail and m % 2:
                    nc.scalar.copy(so, pso)
                else:
                    nc.vector.tensor_copy(so, pso)
                eng = (nc.scalar if m % 2 else nc.sync) if tail else nc.gpsimd
                eng.dma_start(
                    out=outT_d[ts(m, 128), b * N + qc * 512: b * N + (qc + 1) * 512],
                    in_=so,
                )

            # ---- norm: phase a = PSUM drain + reciprocal (no psum
            # held); phase b (scheduled later) = PE broadcast + normalize ----
            rd_pend = {}

            def norm_a(b, qc, otps):
                qs = ts(qc, 512)
                rds = []
                for h in (0, 1):
                    # both halves: d at psum row 0, values rows 1:65
                    stg = stage.tile([65, 512], F16, name="stg")
                    nc.vector.tensor_copy(stg, otps[h][0:65, :])
                    nc.sync.dma_start(out=ot_t[h * 64:(h + 1) * 64, b, qs],
                                      in_=stg[1:65, :])
                    d1 = normp.tile([1, 512], F32, name="d0")
                    r1 = normp.tile([1, 512], F32, name="rd0")
                    r116 = normp.tile([1, 512], F16, name="rd0h")
                    nc.vector.tensor_copy(d1, otps[h][0:1, :])
                    nc.vector.reciprocal_approx_fast(out=r1, in_=d1)
                    nc.vector.tensor_copy(r116, r1)
                    rds.append(r116)
                rd_pend[(b, qc)] = rds

            def norm_b(b, qc):
                qs = ts(qc, 512)
                rds = rd_pend.pop((b, qc))
                bc_ps = ps_c.tile([128, 512], F32, name="ps_c")
                nc.tensor.matmul(bc_ps, sel_t[0:1, 0, :], rds[0],
                                 start=True, stop=False)
                nc.tensor.matmul(bc_ps, sel_t[0:1, 1, :], rds[1],
                                 start=False, stop=True)
                nc.vector.tensor_mul(oc_t[:, b, qs], ot_t[:, b, qs], bc_ps)

            # ---- static filler schedule -------------------------------------
            # unit vocab:  ('Qa',t) ('Qb',t) ('Ka',t) ('Kb',t) ('V',t,sub)
            #              ('OP',g)  -> one outproj m-chunk of group g (8 each)
            sched = {i: [] for i in range(128)}

            def put(slot, *u):
                sched[min(slot, 127)].append(u)

            # Emission-order rule: a filler that WRITES data must be
            # emitted at a slot strictly before the consumer's slot (the
            # tile framework cannot depend on writes emitted later).
            #   Kb_c  <= 4c-1   (scores(0,4c) reads kt chunk c at slot 4c)
            #   V_c,s <= 4c+s+3 (PV(0,kc) runs at slot kc+PV_LAG, after fillers
            #                    of the previous slot)
            put(0, 'V', 0, 0); put(0, 'Ka', 1)
            put(1, 'Kb', 1); put(1, 'V', 0, 1)
            put(2, 'V', 0, 2)
            put(3, 'V', 0, 3); put(3, 'Ka', 2)
            put(4, 'V', 1, 0)
            put(5, 'Kb', 2)
            put(6, 'V', 1, 1)
            put(7, 'V', 1, 2)
            put(8, 'V', 1, 3); put(8, 'Ka', 3)
            put(9, 'V', 2, 0); put(9, 'Kb', 3)
            put(10, 'V', 2, 1)
            put(11, 'V', 2, 2)
            put(12, 'V', 2, 3)
            put(13, 'V', 3, 0)
            put(14, 'V', 3, 1); put(14, 'Qa', 1)
            put(15, 'V', 3, 2); put(15, 'Qb', 1)
            put(16, 'V', 3, 3)
            put(18, 'Qa', 2); put(20, 'Qb', 2)
            put(22, 'Ka', 4); put(24, 'Kb', 4)
            put(26, 'V', 4, 0); put(28, 'V', 4, 1); put(30, 'V', 4, 2)
            put(32, 'Qa', 3); put(34, 'Qb', 3)
            put(36, 'V', 4, 3); put(38, 'V', 5, 0)
            put(40, 'Ka', 5); put(42, 'Kb', 5)
            put(44, 'V', 5, 1); put(46, 'V', 5, 2)
            put(48, 'Qa', 4); put(50, 'Qb', 4)
            put(52, 'V', 5, 3); put(54, 'V', 6, 0)
            put(56, 'Ka', 6); put(58, 'Kb', 6)
            put(60, 'V', 6, 1); put(62, 'V', 6, 2)
            put(64, 'Ka', 7); put(65, 'Kb', 7)
            put(66, 'V', 6, 3); put(68, 'V', 7, 0)
            put(70, 'V', 7, 1); put(72, 'V', 7, 2); put(74, 'V', 7, 3)
            put(77, 'Qa', 5); put(78, 'Qb', 5)
            put(82, 'Qa', 6); put(84, 'Qb', 6)
            put(98, 'Qa', 7); put(100, 'Qb', 7)
            # norm_b(g): after norm_a(g) at slot 16(g+1)+4, recips done
            for g in range(6):
                put(16 * (g + 1) + 8, 'NB', g)
            put(116, 'NB', 6)
            # outproj: group g's oc is written by norm_b(g)
            op_base = [40, 48, 80, 88, 96, 106, 117]
            for g in range(7):
                stride = 1 if g == 6 else 2
                for m in range(8):
                    put(op_base[g] + stride * m, 'OP', g)
            op_done = [0] * 8

            def run_unit(u):
                kind = u[0]
                if kind == 'Qa':
                    proj_a(u[1], wq_t, ('q', u[1]))
                elif kind == 'Qb':
                    proj_b(u[1], wq_t, ('q', u[1]), qt_t, bq_t)
                elif kind == 'Ka':
                    proj_a(u[1], wk_t, ('k', u[1]))
                elif kind == 'Kb':
                    proj_b(u[1], wk_t, ('k', u[1]), kt_t, bk_t)
                elif kind == 'V':
                    proj_v_sub(u[1], u[2])
                elif kind == 'NB':
                    norm_b(u[1] // 4, u[1] % 4)
                elif kind == 'OP':
                    g = u[1]
                    outproj_m(g // 4, g % 4, op_done[g])
                    op_done[g] += 1

            # ---- head: QK projection of chunk 0 ----
            proj_a(0, wq_t, ('q', 0)); proj_b(0, wq_t, ('q', 0), qt_t, bq_t)
            proj_a(0, wk_t, ('k', 0)); proj_b(0, wk_t, ('k', 0), kt_t, bk_t)

            # ---- flat attention pipeline over 128 (group, kc) slots ----
            PV_LAG = 4
            pend = []               # (b, qc, kc, pt) awaiting PV
            cur_otps = None         # PSUM accumulators of the PV-active group
            prev_group = None       # (b, qc, otps) awaiting norm

            def do_pv(pb, pqc, pkc, ppt):
                nonlocal cur_otps, prev_group
                if pkc == 0:
                    cur_otps = [ps_ot.tile([128, 512], F32, name=f"ps_ot{h}")
                                for h in (0, 1)]
                for h in (0, 1):
                    nc.tensor.matmul(
                        cur_otps[h],
                        v_t[:, pb * 16 + pkc, h * 65:h * 65 + 128],
                        ppt[:, ts(h, 512)],
                        start=(pkc == 0), stop=(pkc == 15),
                    )
                if pkc == 15:
                    prev_group = (pb, pqc, cur_otps)

            for i in range(128):
                g, kc = i // 16, i % 16
                b, qc = g // 4, g % 4
                qoff = b * N + qc * 512
                st = ps_st.tile([128, 1024], F32, name="ps_st")
                pt = ptp.tile([128, 1024], F16, name="pt")
                koff = b * N + kc * 128
                for h in (0, 1):
                    lo = h * 64
                    nc.tensor.matmul(
                        st[:, ts(h, 512)],
                        kt_t[lo:lo + 64, koff:koff + 128],
                        qt_t[lo:lo + 64, qoff:qoff + 512],
                        start=True, stop=True,
                    )
                nc.scalar.activation(
                    pt, st, mybir.ActivationFunctionType.Exp, scale=SCALE,
                )
                # norm(g-1) before this slot's PV so the accumulator pool's
                # re-allocation (at pkc==0) happens after all of its reads
                if prev_group is not None:
                    ng = prev_group
                    prev_group = None
                    norm_a(ng[0], ng[1], ng[2])
                if len(pend) >= PV_LAG:
                    do_pv(*pend.pop(0))
                pend.append((b, qc, kc, pt))
                for u in sched[i]:
                    run_unit(u)

            # ---- epilogue ----
            while pend:
                if prev_group is not None:
                    ng = prev_group
                    prev_group = None
                    norm_a(ng[0], ng[1], ng[2])
                do_pv(*pend.pop(0))
            # keep the PE busy (HAM warm) while the final norm chain runs
            psd2 = ps_c.tile([64, 64], F32, name="ps_c")
            for i in range(58):
                nc.tensor.matmul(psd2, dum_t, dum_t,
                                 start=(i == 0), stop=(i == 57))
            norm_a(1, 3, cur_otps)
            for g in range(7):
                for m in range(8 - op_done[g]):
                    run_unit(('OP', g))
            norm_b(1, 3)
            for m in range(8):
                outproj_m(1, 3, m, tail=True)

    nc.compile()
    return nc


def kernel(x, Wq, bq, Wk, bk, Wv, bv, Wp, bp,
           lambda_q1, lambda_k1, lambda_q2, lambda_k2):
    x = np.asarray(x, dtype=np.float32)
    Wq, Wk, Wv, Wp = [np.asarray(w, dtype=np.float32) for w in (Wq, Wk, Wv, Wp)]
    bq, bk, bv, bp = [np.asarray(v, dtype=np.float32) for v in (bq, bk, bv, bp)]

    l1 = np.exp(np.minimum(
        (np.asarray(lambda_q1, np.float32) * np.asarray(lambda_k1, np.float32))
        .sum((-1, -2)), 5.0))
    l2 = np.exp(np.minimum(
        (np.asarray(lambda_q2, np.float32) * np.asarray(lambda_k2, np.float32))
        .sum((-1, -2)), 5.0))
    lv = np.float32((l1 - l2 + np.float32(LAMBDA_INIT)).mean())

    xT = x.reshape(T, EMBED).T.astype(np.float16)
    xT = np.ascontiguousarray(xT.reshape(8, 128, 8, 512).transpose(1, 2, 0, 3))

    if _compiled[0] is None:
        _compiled[0] = _build()
    nc = _compiled[0]

    in_maps = []
    for p in range(NCORES):
        r1 = slice(p * HD, (p + 1) * HD)          # head p rows/cols
        r2 = slice((8 + p) * HD, (9 + p) * HD)    # head p+8 rows/cols
        wq_p = np.concatenate([Wq[r1], Wq[r2]], 0).T      # [1024, 128]
        wk_p = np.concatenate([Wk[r1], Wk[r2]], 0).T
        wv_p = np.concatenate([Wv[r1], Wv[r2]], 0).T
        wpt1 = Wp[:, r1].T                                 # [64, 1024]
        wpt2 = Wp[:, r2].T
        wcomb = np.concatenate([wpt1, wpt2 - lv * wpt1], 0)  # [128, 1024]
        bva = np.concatenate(
            [[1.0], bv[r1], [1.0], bv[r2]]).astype(np.float32)[None, :]
        in_maps.append({
            "xT": xT,
            "wqk": np.ascontiguousarray(np.concatenate([
                wq_p.reshape(8, 128, 128).transpose(1, 0, 2),
                wk_p.reshape(8, 128, 128).transpose(1, 0, 2)], axis=1)
                .astype(np.float16)),
            "wv": np.ascontiguousarray(
                wv_p.reshape(8, 128, 128).transpose(1, 0, 2).astype(np.float16)),
            "wcomb": np.ascontiguousarray(wcomb.astype(np.float16)),
            "bqk": np.ascontiguousarray(np.stack([
                np.concatenate([bq[r1], bq[r2]]),
                np.concatenate([bk[r1], bk[r2]])], axis=1).astype(np.float32)),
            "bvaug": np.ascontiguousarray(bva),
        })

    res = run_bass_kernel_spmd(
        nc, in_maps, core_ids=list(range(NCORES)), trace=TRACE,
    )
    LAST_RESULT[0] = res

    outT = res.results[0]["outT"].astype(np.float64)
    for c in range(1, NCORES):
        outT += res.results[c]["outT"]
    out = outT.T.reshape(B, N, EMBED).astype(np.float32) + bp[None, None, :]
    return out
